# revision 16
# baseline (speedup 1.0000x reference)
"""Causal self-attention (GQA + RoPE) on 8 Trainium2 NeuronCores.

Sharding: head-parallel. Core c owns heads [4c, 4c+4) — exactly one KV head —
over both batches flattened to 4096 rows. bf16 matmuls throughout (PSUM fp32
accumulate). Attention runs in transposed layout (keys on partitions, queries
on free dim; softmax denominators via a ones-column appended to V).

Output rows are interleaved across cores (dest r owns query blocks qb=r and
qb=8+r) so the y redistribution splits into two AllToAlls: the first (batch 0)
is issued mid-attention and overlaps with batch-1 attention compute; only the
second is (briefly) exposed. Wo is preloaded to SBUF during attention. Each
core then computes the full o_proj for its 512 interleaved rows.
"""

import numpy as np
import ml_dtypes

import concourse.bass as bass
import concourse.mybir as mybir
import concourse.tile as tile
from concourse.bass_utils import run_bass_kernel_spmd

F32 = mybir.dt.float32
BF = mybir.dt.bfloat16
AF = mybir.ActivationFunctionType
MUL = mybir.AluOpType.mult
NPBF = ml_dtypes.bfloat16

N_CORES = 8
B, T, C = 2, 2048, 2048
H, KV, D = 32, 8, 64
TT = B * T                     # 4096 flattened rows
HL = H // N_CORES              # 4 local heads
ROPE_THETA = 500000.0

N_TC = TT // 512               # 8 projection column chunks
N_QB = TT // 256               # 16 query blocks of 256
N_KC = TT // 128               # 32 key chunks of 128
VW = 68                        # padded per-chunk stride in the packed V tile


def _split_waits(nc):
    """This container's walrus accepts at most ONE sync-wait per instruction.

    Move extra waits onto NoOp carriers immediately before the instruction on
    the same engine (engine executes in order, so blocking semantics hold)."""
    for f in nc.m.functions:
        for blk in f.blocks:
            insts = list(blk.instructions)
            out = []
            changed = False
            for inst in insts:
                si = inst.sync_info
                if si is not None and len(si.on_wait) > 1:
                    changed = True
                    waits = list(si.on_wait)
                    for w in waits[:-1]:
                        nop = mybir.InstNoOp(
                            name=nc.get_next_instruction_name(), ins=[], outs=[]
                        )
                        nop.engine = inst.engine
                        nop.sync_info = mybir.SyncInfo(on_wait=[w], on_update=[])
                        out.append(nop)
                    inst.sync_info = mybir.SyncInfo(
                        on_wait=[waits[-1]], on_update=list(si.on_update)
                    )
                out.append(inst)
            if changed:
                blk.instructions = out


def build_program():
    nc = bass.Bass("TRN2", target_bir_lowering=False, debug=False,
                   num_devices=N_CORES)

    XT = nc.dram_tensor("XT", [C, TT], BF, kind="ExternalInput").ap()
    WQS = nc.dram_tensor("WQS", [C, HL * D], BF, kind="ExternalInput").ap()
    WKV = nc.dram_tensor("WKV", [C, 2 * D], BF, kind="ExternalInput").ap()
    WOB = nc.dram_tensor("WOB", [C, C], BF, kind="ExternalInput").ap()
    COS = nc.dram_tensor("COS", [128, TT], BF, kind="ExternalInput").ap()
    SIN = nc.dram_tensor("SIN", [128, TT], BF, kind="ExternalInput").ap()
    MA = nc.dram_tensor("MA", [128, 256], BF, kind="ExternalInput").ap()
    MB = nc.dram_tensor("MB", [128, 256], BF, kind="ExternalInput").ap()
    ONEC = nc.dram_tensor("ONEC", [128, 1], BF, kind="ExternalInput").ap()
    R2T = nc.dram_tensor("R2T", [128, 128], BF, kind="ExternalInput").ap()
    IDN = nc.dram_tensor("IDN", [128, 64], BF, kind="ExternalInput").ap()
    EALL = nc.dram_tensor("EALL", [32, 2048], BF, kind="ExternalInput").ap()
    OUTT = nc.dram_tensor("OUTT", [C, TT // N_CORES], F32,
                          kind="ExternalOutput").ap()

    DSI = nc.dram_tensor("cc_warm_in", [N_CORES, 16], BF).ap()
    DSO = nc.dram_tensor("cc_warm_out", [N_CORES, 16], BF).ap()
    A2AI = nc.dram_tensor("a2a_in", [2, N_CORES, HL * D + 4, 256], BF).ap()
    A2AO = nc.dram_tensor("a2a_out", [2, N_CORES, HL * D + 4, 256], BF).ap()

    with tile.TileContext(nc) as tc, nc.allow_low_precision(reason="bf16"):
        with tc.tile_pool(name="pers", bufs=1) as pers, \
             tc.tile_pool(name="wq_sb", bufs=1) as wq_pool, \
             tc.tile_pool(name="wo_sb", bufs=1) as wo_pool:
            # single big weight DMAs (per-queue outstanding limits serialize
            # many small loads); wq on sync, wkv on scalar, tables on gpsimd
            wqcat = wq_pool.tile([128, 16 * HL * D], BF)
            wqv = WQS.rearrange("(cb p) q -> p cb q", p=128)
            nc.sync.dma_start(
                wqcat[:].rearrange("p (cb q) -> p cb q", cb=16)[:, 0:2, :],
                wqv[:, 0:2, :])
            nc.sync.dma_start(
                wqcat[:].rearrange("p (cb q) -> p cb q", cb=16)[:, 2:16, :],
                wqv[:, 2:16, :])
            wq_sb = [wqcat[:, cb * HL * D:(cb + 1) * HL * D]
                     for cb in range(16)]
            wkvcat = wq_pool.tile([128, 16 * 2 * D], BF)
            nc.scalar.dma_start(
                wkvcat[:], WKV.rearrange("(cb p) q -> p cb q", p=128))
            wkv_sb = [wkvcat[:, cb * 2 * D:(cb + 1) * 2 * D]
                      for cb in range(16)]

            cos = pers.tile([128, TT], BF)
            sin = pers.tile([128, TT], BF)
            r2t = pers.tile([128, 128], BF)
            ident = pers.tile([128, 64], BF)
            onec = pers.tile([128, 1], BF)
            ma = pers.tile([128, 256], BF)
            mb = pers.tile([128, 256], BF)
            eall = pers.tile([32, 2048], BF)
            for dst, srcap in ((r2t, R2T), (ident, IDN), (onec, ONEC),
                               (ma, MA), (mb, MB), (eall, EALL)):
                nc.gpsimd.dma_start(dst[:], srcap[:])
            # tiny collective to absorb first-collective stream warmup
            nc.gpsimd.collective_compute(
                "AllToAll", mybir.AluOpType.bypass,
                replica_groups=[list(range(N_CORES))], ins=[DSI[:]],
                outs=[DSO[:]])

            qTa = pers.tile([128, 2 * TT], BF)
            kT = pers.tile([128, TT], BF)
            vp = pers.tile([128, N_KC * VW], BF)

            wob = [wo_pool.tile([128, C], BF, tag=f"wo{cc}", name=f"wo{cc}")
                   for cc in range(16)]
            den = wo_pool.tile([32, 512], BF)
            den_f = wo_pool.tile([32, 512], F32)
            rec = wo_pool.tile([32, 512], F32)
            orhs = [wo_pool.tile([128, 512], BF, tag=f"or{cc}",
                                 name=f"or{cc}") for cc in range(16)]
            rec_bf0 = wo_pool.tile([32, 256], BF)
            rawg = [wo_pool.tile([128, 16 * 256], BF, tag=f"rawg{g}",
                                 name=f"rawg{g}") for g in range(2)]

            # ============== Phase P: projections + RoPE ==============
            with tc.tile_pool(name="xt_sb", bufs=2) as xt_pool, \
                 tc.tile_pool(name="q_ps", bufs=2, space="PSUM") as qps, \
                 tc.tile_pool(name="kv_ps", bufs=2, space="PSUM") as kvps, \
                 tc.tile_pool(name="rt_ps", bufs=1, space="PSUM") as rtps, \
                 tc.tile_pool(name="proj_tmp", bufs=2) as ptmp:
                for tcb in range(N_TC):
                    tsl = slice(tcb * 512, (tcb + 1) * 512)
                    xtc = xt_pool.tile([128, 16 * 512], BF, tag="xtc",
                                       name="xtc")
                    for hf in range(2):
                        eng = nc.sync if hf == 0 else nc.scalar
                        eng.dma_start(
                            xtc[:, hf * 4096:(hf + 1) * 4096],
                            XT[hf * 1024:(hf + 1) * 1024, tsl].rearrange(
                                "(cb p) q -> p cb q", p=128))
                    xt = [xtc[:, cb * 512:(cb + 1) * 512] for cb in range(16)]
                    if tcb == 0:
                        # cos/sin ride after the first x tile: keeps the HBM
                        # burst at startup prioritized for the matmul pipeline
                        nc.vector.tensor_copy(cos[0:1, 0:1], xtc[0:1, 0:1])
                        nc.vector.tensor_copy(sin[0:1, 0:1], xtc[0:1, 0:1])
                        nc.gpsimd.dma_start(cos[:], COS[:])
                        nc.gpsimd.dma_start(sin[:], SIN[:])
                    qp = [qps.tile([128, 512], F32, tag=f"qp{t}",
                                   name=f"qp{t}") for t in range(2)]
                    kvp = kvps.tile([128, 512], F32, tag="kvp")
                    for cb in range(16):
                        st = dict(start=(cb == 0), stop=(cb == 15))
                        for t in range(2):
                            nc.tensor.matmul(
                                qp[t][:], wq_sb[cb][:, t * 128:(t + 1) * 128],
                                xt[cb], **st)
                        nc.tensor.matmul(kvp[:], wkv_sb[cb][:], xt[cb], **st)
                    # RoPE on q tiles
                    for t in range(2):
                        qraw = ptmp.tile([128, 512], BF, tag="qraw")
                        nc.vector.tensor_copy(qraw[:], qp[t][:])
                        rot = rtps.tile([128, 512], F32, tag="rot")
                        nc.tensor.matmul(rot[:], r2t[:], qraw[:],
                                         start=True, stop=True)
                        qsl_t = slice(t * TT + tcb * 512,
                                      t * TT + (tcb + 1) * 512)
                        nc.vector.tensor_mul(qTa[:, qsl_t], qraw[:],
                                             cos[:, tsl])
                        t2 = ptmp.tile([128, 512], BF, tag="t2")
                        nc.vector.tensor_mul(t2[:], rot[:], sin[:, tsl])
                        nc.vector.tensor_add(qTa[:, qsl_t], qTa[:, qsl_t],
                                             t2[:])
                    # v (rows 64:128 of kvp) -> natural layout via PE
                    vraw = ptmp.tile([128, 512], BF, tag="vraw")
                    nc.vector.tensor_copy(vraw[64:128, :], kvp[64:128, :])
                    for r in range(4):
                        i = tcb * 4 + r
                        tp = rtps.tile([128, 64], BF, tag="tp")
                        nc.tensor.transpose(
                            tp[:], vraw[64:128, r * 128:(r + 1) * 128],
                            ident[64:128, :])
                        nc.vector.tensor_copy(vp[:, i * VW:i * VW + 64], tp[:])
                        nc.vector.tensor_copy(
                            vp[:, i * VW + 64:i * VW + 65], onec[:])
                    # RoPE on k (rows 0:64 of kvp)
                    kraw = ptmp.tile([64, 512], BF, tag="kraw")
                    nc.vector.tensor_copy(kraw[:], kvp[0:64, :])
                    krot = rtps.tile([64, 512], F32, tag="rot")
                    nc.tensor.matmul(krot[:], r2t[0:64, 0:64], kraw[:],
                                     start=True, stop=True)
                    nc.vector.tensor_mul(kT[0:64, tsl], kraw[:],
                                          cos[0:64, tsl])
                    k2 = ptmp.tile([64, 512], BF, tag="k2")
                    nc.vector.tensor_mul(k2[:], krot[:], sin[0:64, tsl])
                    nc.vector.tensor_add(kT[0:64, tsl], kT[0:64, tsl], k2[:])
                    # mirror k to partitions 64:127 for row-packed scores
                    nc.sync.dma_start(kT[64:128, tsl], kT[0:64, tsl])

            # ================== Phase A: attention ==================
            groups = [list(range(N_CORES))]
            with tc.tile_pool(name="sc_ps", bufs=2, space="PSUM") as scps, \
                 tc.tile_pool(name="av_ps", bufs=2, space="PSUM") as avps, \
                 tc.tile_pool(name="ex_sb", bufs=3) as exp_pool, \
                 tc.tile_pool(name="sg_sb", bufs=4) as sg_pool:
                for qb in range(N_QB):
                    qsl = slice(qb * 256, (qb + 1) * 256)
                    g, rr = qb // 8, qb % 8
                    wb = qb % 8
                    base_kc = (qb // 8) * 16
                    nch = 2 * wb + 2
                    y_lo = avps.tile([65, 512], F32, tag="ylo")
                    y_hi = avps.tile([65, 512], F32, tag="yhi")
                    for ck in range(nch):
                        kc = base_kc + ck
                        ksl = slice(kc * 128, (kc + 1) * 128)
                        sc = scps.tile([128, 1024], F32, tag="sc")
                        qv = qTa[:].rearrange("p (t n) -> p t n", t=2)[:, :, qsl]
                        nc.tensor.matmul(sc[:, 0:512], kT[0:64, ksl],
                                         qv[0:64], start=True, stop=True)
                        nc.tensor.matmul(sc[:, 512:1024], kT[64:128, ksl],
                                         qv[64:128], start=True, stop=True)
                        ex = exp_pool.tile([128, 1024], BF, tag="ex")
                        nc.scalar.activation(ex[:], sc[:], AF.Exp, scale=0.125)
                        if ck == nch - 2:
                            for blk in range(4):
                                bsl = slice(blk * 256, (blk + 1) * 256)
                                nc.vector.tensor_mul(ex[:, bsl], ex[:, bsl],
                                                     ma[:])
                        if ck == nch - 1:
                            for blk in range(4):
                                bsl = slice(blk * 256, (blk + 1) * 256)
                                nc.vector.tensor_mul(ex[:, bsl], ex[:, bsl],
                                                     mb[:])
                        st = dict(start=(ck == 0), stop=(ck == nch - 1))
                        vsl = vp[:, kc * VW:kc * VW + 65]
                        nc.tensor.matmul(y_lo[:], vsl, ex[:, 0:512], **st)
                        nc.tensor.matmul(y_hi[:], vsl, ex[:, 512:1024], **st)
                    # stage this block's y + denominators straight to the
                    # a2a buffer (dest rr, chunk g); normalization happens on
                    # the receiver after the AllToAll
                    sg_lo = sg_pool.tile([65, 512], BF, tag="sglo")
                    sg_hi = sg_pool.tile([65, 512], BF, tag="sghi")
                    nc.vector.tensor_copy(sg_lo[:], y_lo[:])
                    nc.vector.tensor_copy(sg_hi[:], y_hi[:])
                    blocks = A2AI[g, rr, 0:256].rearrange(
                        "(c k p) q -> k p c q", c=2, k=2, p=64)
                    dens = A2AI[g, rr, 256:260].rearrange(
                        "(a b) q -> b a q", a=2, b=2)
                    nc.sync.dma_start(
                        blocks[0], sg_lo[0:64, :].rearrange(
                            "p (c q) -> p c q", c=2))
                    nc.gpsimd.dma_start(
                        blocks[1], sg_hi[0:64, :].rearrange(
                            "p (c q) -> p c q", c=2))
                    nc.sync.dma_start(
                        dens[0], sg_lo[64:65, :].rearrange(
                            "p (a q) -> p a q", a=2))
                    nc.gpsimd.dma_start(
                        dens[1], sg_hi[64:65, :].rearrange(
                            "p (a q) -> p a q", a=2))
                    # spread Wo preload over qb 0..11 so every tile lands
                    # before A2A#1; the tiny copy makes each load wait for
                    # this qb (8 SW DMA queues run concurrently, so without
                    # it the loads all issue at startup and steal HBM
                    # bandwidth from the x pipeline)
                    if qb < 12:
                        ccs = [qb] if qb < 8 else [8 + 2 * (qb - 8),
                                                   9 + 2 * (qb - 8)]
                        for cc in ccs:
                            nc.vector.tensor_copy(wob[cc][0:1, 0:1],
                                                  sg_lo[0:1, 0:1])
                            nc.gpsimd.dma_start(
                                wob[cc][:], WOB[cc * 128:(cc + 1) * 128, :])
                    if qb == 13:
                        # batch-0 y normalization, fully off the critical path
                        nc.gpsimd.dma_start(den[:, 0:256],
                                            A2AO[0, :, 256:260, :])
                        nc.vector.tensor_copy(den_f[:, 0:256], den[:, 0:256])
                        nc.vector.reciprocal(rec[:, 0:256], den_f[:, 0:256])
                        nc.vector.tensor_copy(rec_bf0[:], rec[:, 0:256])
                        rgv = rawg[0][:].rearrange("p (i t q) -> p i t q",
                                                   i=8, t=2)
                        for tb in range(2):
                            nc.gpsimd.dma_start(
                                rgv[:, :, tb, :],
                                A2AO[0, :, tb * 128:(tb + 1) * 128,
                                     :].rearrange("i p q -> p i q"))
                    if qb == 7:
                        nc.gpsimd.collective_compute(
                            "AllToAll", mybir.AluOpType.bypass,
                            replica_groups=groups,
                            ins=[A2AI[0]], outs=[A2AO[0]])
                nc.gpsimd.collective_compute(
                    "AllToAll", mybir.AluOpType.bypass,
                    replica_groups=groups, ins=[A2AI[1]], outs=[A2AO[1]])

            # ==================== Phase O: o_proj ====================
            with tc.tile_pool(name="o_ps", bufs=3, space="PSUM") as ops_pool, \
                 tc.tile_pool(name="bc_ps", bufs=2, space="PSUM") as bcps, \
                 tc.tile_pool(name="osb_sb", bufs=3) as osb_pool:
                nc.scalar.dma_start(den[:, 256:512], A2AO[1, :, 256:260, :])
                rgv = rawg[1][:].rearrange("p (i t q) -> p i t q", i=8, t=2)
                for tb in range(2):
                    nc.scalar.dma_start(
                        rgv[:, :, tb, :],
                        A2AO[1, :, tb * 128:(tb + 1) * 128, :].rearrange(
                            "i p q -> p i q"))
                rec_bf = osb_pool.tile([32, 512], BF, tag="recbf", bufs=1)
                nc.vector.tensor_copy(den_f[:, 256:512], den[:, 256:512])
                nc.vector.reciprocal(rec[:, 256:512], den_f[:, 256:512])
                nc.vector.tensor_copy(rec_bf[:, 256:512], rec[:, 256:512])
                for g in range(2):
                    gsl = slice(g * 256, (g + 1) * 256)
                    rsrc = rec_bf0 if g == 0 else rec_bf
                    for occ in range(16):
                        bc = bcps.tile([128, 256], F32, tag="bc")
                        nc.tensor.matmul(
                            bc[:], eall[:, occ * 128:(occ + 1) * 128],
                            rsrc[:, 0:256] if g == 0 else rec_bf[:, gsl],
                            start=True, stop=True)
                        nc.vector.scalar_tensor_tensor(
                            orhs[occ][:, gsl], bc[:], 1.0,
                            rawg[g][:, occ * 256:(occ + 1) * 256], MUL, MUL)
                # per-half m-loops: the batch-0 half starts right after the
                # last AV matmul, overlapping A2A#1 + batch-1 normalization
                for g in range(2):
                    gsl = slice(g * 256, (g + 1) * 256)
                    for m in range(16):
                        op = ops_pool.tile([128, 256], F32, tag="op")
                        for cc in range(16):
                            nc.tensor.matmul(
                                op[:], wob[cc][:, m * 128:(m + 1) * 128],
                                orhs[cc][:, gsl],
                                start=(cc == 0), stop=(cc == 15))
                        osb = osb_pool.tile([128, 256], F32, tag="osb")
                        nc.scalar.activation(osb[:], op[:], AF.Copy)
                        eng = nc.sync if m % 2 == 0 else nc.scalar
                        eng.dma_start(
                            OUTT[m * 128:(m + 1) * 128, gsl], osb[:])

    _split_waits(nc)
    return nc


def host_inputs(x, Wq, Wk, Wv, Wo):
    """Per-core input maps (host-side sharding + precomputed tables)."""
    x = np.asarray(x, np.float32)
    Wq = np.asarray(Wq, np.float32)
    Wk = np.asarray(Wk, np.float32)
    Wv = np.asarray(Wv, np.float32)
    Wo = np.asarray(Wo, np.float32)

    xt = np.ascontiguousarray(x.reshape(TT, C).T).astype(NPBF)   # [C, TT]

    inv_freq = (1.0 / (ROPE_THETA ** (np.arange(0, D, 2) / D))).astype(np.float64)
    pos = (np.arange(TT) % T).astype(np.float64)
    ang = pos[None, :] * inv_freq[np.arange(128) % 32][:, None]   # [128, TT]
    cos_t = np.cos(ang).astype(NPBF)
    sin_t = np.sin(ang).astype(NPBF)

    ki = np.arange(128)[:, None]
    qf = np.arange(256)[None, :]
    ma = (ki <= qf).astype(NPBF)
    mb = (ki + 128 <= qf).astype(NPBF)

    R = np.zeros((64, 64), np.float32)
    for mrow in range(32):
        R[mrow, mrow + 32] = -1.0
        R[mrow + 32, mrow] = 1.0
    R2 = np.zeros((128, 128), np.float32)
    R2[0:64, 0:64] = R
    R2[64:128, 64:128] = R
    r2t = np.ascontiguousarray(R2.T).astype(NPBF)

    wob = np.empty((C, C), np.float32)
    row = 0
    for i in range(N_CORES):
        for t in range(2):
            for h in (4 * i + t, 4 * i + t + 2):
                wob[row:row + 64, :] = Wo[h * 64:(h + 1) * 64, :]
                row += 64
    wob = wob.astype(NPBF)

    eall = np.zeros((32, 2048), np.float32)
    for cc in range(16):
        i, t = cc // 2, cc % 2
        eall[i * 4 + 2 * t, cc * 128:cc * 128 + 64] = 1.0
        eall[i * 4 + 2 * t + 1, cc * 128 + 64:cc * 128 + 128] = 1.0
    eall = eall.astype(NPBF)

    maps = []
    for c in range(N_CORES):
        wqs = np.empty((C, HL * D), np.float32)
        col = 0
        for t in range(2):
            for h in (4 * c + t, 4 * c + t + 2):
                wqs[:, col:col + 64] = Wq[:, h * 64:(h + 1) * 64]
                col += 64
        wkv = np.concatenate(
            [Wk[:, c * 64:(c + 1) * 64], Wv[:, c * 64:(c + 1) * 64]], axis=1)
        maps.append({
            "XT": xt,
            "WQS": wqs.astype(NPBF),
            "WKV": np.ascontiguousarray(wkv).astype(NPBF),
            "WOB": wob,
            "COS": cos_t,
            "SIN": sin_t,
            "MA": ma,
            "MB": mb,
            "ONEC": np.ones((128, 1), NPBF),
            "R2T": r2t,
            "IDN": np.concatenate([np.zeros((64, 64), np.float32),
                                   np.eye(64, dtype=np.float32)],
                                  axis=0).astype(NPBF),
            "EALL": eall,
        })
    return maps


def assemble_output(results, dtype=np.float32):
    out = np.empty((TT, C), dtype)
    for c in range(N_CORES):
        o = results[c]["OUTT"]  # [C, 512]; cols 0:256 = qb c, 256:512 = qb 8+c
        out[c * 256:(c + 1) * 256, :] = o[:, 0:256].T
        out[2048 + c * 256:2048 + (c + 1) * 256, :] = o[:, 256:512].T
    return out.reshape(B, T, C)


_NC_CACHE = None


def get_program():
    global _NC_CACHE
    if _NC_CACHE is None:
        _NC_CACHE = build_program()
    return _NC_CACHE


def kernel(x, Wq, Wk, Wv, Wo):
    nc = get_program()
    maps = host_inputs(x, Wq, Wk, Wv, Wo)
    res = run_bass_kernel_spmd(nc, maps, list(range(N_CORES)))
    return assemble_output(res.results, np.asarray(x).dtype)


if __name__ == "__main__":
    rng = np.random.default_rng(0)
    s = 1.0 / np.sqrt(C)
    x = rng.standard_normal((B, T, C), dtype=np.float32)
    Wq = rng.standard_normal((C, C), dtype=np.float32) * s
    Wk = rng.standard_normal((C, KV * D), dtype=np.float32) * s
    Wv = rng.standard_normal((C, KV * D), dtype=np.float32) * s
    Wo = rng.standard_normal((C, C), dtype=np.float32) * s
    y = kernel(x=x, Wq=Wq, Wk=Wk, Wv=Wv, Wo=Wo)
    print("out", y.shape, y.dtype, float(np.abs(y).max()))


# revision 17
# speedup vs baseline: 1.0034x; 1.0034x over previous
"""Causal self-attention (GQA + RoPE) on 8 Trainium2 NeuronCores.

Sharding: head-parallel. Core c owns heads [4c, 4c+4) — exactly one KV head —
over both batches flattened to 4096 rows. bf16 matmuls throughout (PSUM fp32
accumulate). Attention runs in transposed layout (keys on partitions, queries
on free dim; softmax denominators via a ones-column appended to V).

Output rows are interleaved across cores (dest r owns query blocks qb=r and
qb=8+r) so the y redistribution splits into two AllToAlls: the first (batch 0)
is issued mid-attention and overlaps with batch-1 attention compute; only the
second is (briefly) exposed. Wo is preloaded to SBUF during attention. Each
core then computes the full o_proj for its 512 interleaved rows.
"""

import numpy as np
import ml_dtypes

import concourse.bass as bass
import concourse.mybir as mybir
import concourse.tile as tile
from concourse.bass_utils import run_bass_kernel_spmd

F32 = mybir.dt.float32
BF = mybir.dt.bfloat16
AF = mybir.ActivationFunctionType
MUL = mybir.AluOpType.mult
NPBF = ml_dtypes.bfloat16

N_CORES = 8
B, T, C = 2, 2048, 2048
H, KV, D = 32, 8, 64
TT = B * T                     # 4096 flattened rows
HL = H // N_CORES              # 4 local heads
ROPE_THETA = 500000.0

N_TC = TT // 512               # 8 projection column chunks
N_QB = TT // 256               # 16 query blocks of 256
N_KC = TT // 128               # 32 key chunks of 128
VW = 68                        # padded per-chunk stride in the packed V tile


def _split_waits(nc):
    """This container's walrus accepts at most ONE sync-wait per instruction.

    Move extra waits onto NoOp carriers immediately before the instruction on
    the same engine (engine executes in order, so blocking semantics hold)."""
    for f in nc.m.functions:
        for blk in f.blocks:
            insts = list(blk.instructions)
            out = []
            changed = False
            for inst in insts:
                si = inst.sync_info
                if si is not None and len(si.on_wait) > 1:
                    changed = True
                    waits = list(si.on_wait)
                    for w in waits[:-1]:
                        nop = mybir.InstNoOp(
                            name=nc.get_next_instruction_name(), ins=[], outs=[]
                        )
                        nop.engine = inst.engine
                        nop.sync_info = mybir.SyncInfo(on_wait=[w], on_update=[])
                        out.append(nop)
                    inst.sync_info = mybir.SyncInfo(
                        on_wait=[waits[-1]], on_update=list(si.on_update)
                    )
                out.append(inst)
            if changed:
                blk.instructions = out


def build_program():
    nc = bass.Bass("TRN2", target_bir_lowering=False, debug=False,
                   num_devices=N_CORES)

    XT = nc.dram_tensor("XT", [C, TT], BF, kind="ExternalInput").ap()
    WQS = nc.dram_tensor("WQS", [C, HL * D], BF, kind="ExternalInput").ap()
    WKV = nc.dram_tensor("WKV", [C, 2 * D], BF, kind="ExternalInput").ap()
    WOB = nc.dram_tensor("WOB", [C, C], BF, kind="ExternalInput").ap()
    COS = nc.dram_tensor("COS", [128, TT], BF, kind="ExternalInput").ap()
    SIN = nc.dram_tensor("SIN", [128, TT], BF, kind="ExternalInput").ap()
    MA = nc.dram_tensor("MA", [128, 256], BF, kind="ExternalInput").ap()
    MB = nc.dram_tensor("MB", [128, 256], BF, kind="ExternalInput").ap()
    ONEC = nc.dram_tensor("ONEC", [128, 1], BF, kind="ExternalInput").ap()
    R2T = nc.dram_tensor("R2T", [128, 128], BF, kind="ExternalInput").ap()
    IDN = nc.dram_tensor("IDN", [128, 64], BF, kind="ExternalInput").ap()
    EALL = nc.dram_tensor("EALL", [32, 2048], BF, kind="ExternalInput").ap()
    OUTT = nc.dram_tensor("OUTT", [C, TT // N_CORES], F32,
                          kind="ExternalOutput").ap()

    DSI = nc.dram_tensor("cc_warm_in", [N_CORES, 16], BF).ap()
    DSO = nc.dram_tensor("cc_warm_out", [N_CORES, 16], BF).ap()
    A2AI = nc.dram_tensor("a2a_in", [2, N_CORES, HL * D + 4, 256], BF).ap()
    A2AO = nc.dram_tensor("a2a_out", [2, N_CORES, HL * D + 4, 256], BF).ap()

    with tile.TileContext(nc) as tc, nc.allow_low_precision(reason="bf16"):
        with tc.tile_pool(name="pers", bufs=1) as pers, \
             tc.tile_pool(name="wq_sb", bufs=1) as wq_pool, \
             tc.tile_pool(name="wo_sb", bufs=1) as wo_pool:
            # single big weight DMAs (per-queue outstanding limits serialize
            # many small loads); wq on sync, wkv on scalar, tables on gpsimd
            wqcat = wq_pool.tile([128, 16 * HL * D], BF)
            wqv = WQS.rearrange("(cb p) q -> p cb q", p=128)
            nc.sync.dma_start(
                wqcat[:].rearrange("p (cb q) -> p cb q", cb=16)[:, 0:2, :],
                wqv[:, 0:2, :])
            nc.sync.dma_start(
                wqcat[:].rearrange("p (cb q) -> p cb q", cb=16)[:, 2:16, :],
                wqv[:, 2:16, :])
            wq_sb = [wqcat[:, cb * HL * D:(cb + 1) * HL * D]
                     for cb in range(16)]
            wkvcat = wq_pool.tile([128, 16 * 2 * D], BF)
            nc.scalar.dma_start(
                wkvcat[:], WKV.rearrange("(cb p) q -> p cb q", p=128))
            wkv_sb = [wkvcat[:, cb * 2 * D:(cb + 1) * 2 * D]
                      for cb in range(16)]

            cos = pers.tile([128, TT], BF)
            sin = pers.tile([128, TT], BF)
            r2t = pers.tile([128, 128], BF)
            ident = pers.tile([128, 64], BF)
            onec = pers.tile([128, 1], BF)
            ma = pers.tile([128, 256], BF)
            mb = pers.tile([128, 256], BF)
            eall = pers.tile([32, 2048], BF)
            for dst, srcap in ((r2t, R2T), (ident, IDN), (onec, ONEC),
                               (ma, MA), (mb, MB), (eall, EALL)):
                nc.gpsimd.dma_start(dst[:], srcap[:])
            # tiny collective to absorb first-collective stream warmup
            nc.gpsimd.collective_compute(
                "AllToAll", mybir.AluOpType.bypass,
                replica_groups=[list(range(N_CORES))], ins=[DSI[:]],
                outs=[DSO[:]])

            qTa = pers.tile([128, 2 * TT], BF)
            kT = pers.tile([128, TT], BF)
            vp = pers.tile([128, N_KC * VW], BF)

            wob = [wo_pool.tile([128, C], BF, tag=f"wo{cc}", name=f"wo{cc}")
                   for cc in range(16)]
            den = wo_pool.tile([32, 512], BF)
            den_f = wo_pool.tile([32, 512], F32)
            rec = wo_pool.tile([32, 512], F32)
            orhs = [wo_pool.tile([128, 512], BF, tag=f"or{cc}",
                                 name=f"or{cc}") for cc in range(16)]
            rec_bf0 = wo_pool.tile([32, 256], BF)
            rawg = [wo_pool.tile([128, 16 * 256], BF, tag=f"rawg{g}",
                                 name=f"rawg{g}") for g in range(2)]

            # ============== Phase P: projections + RoPE ==============
            with tc.tile_pool(name="xt_sb", bufs=2) as xt_pool, \
                 tc.tile_pool(name="q_ps", bufs=2, space="PSUM") as qps, \
                 tc.tile_pool(name="kv_ps", bufs=2, space="PSUM") as kvps, \
                 tc.tile_pool(name="rt_ps", bufs=1, space="PSUM") as rtps, \
                 tc.tile_pool(name="proj_tmp", bufs=2) as ptmp:
                for tcb in range(N_TC):
                    tsl = slice(tcb * 512, (tcb + 1) * 512)
                    xtc = xt_pool.tile([128, 16 * 512], BF, tag="xtc",
                                       name="xtc")
                    for hf in range(2):
                        eng = nc.sync if hf == 0 else nc.scalar
                        eng.dma_start(
                            xtc[:, hf * 4096:(hf + 1) * 4096],
                            XT[hf * 1024:(hf + 1) * 1024, tsl].rearrange(
                                "(cb p) q -> p cb q", p=128))
                    xt = [xtc[:, cb * 512:(cb + 1) * 512] for cb in range(16)]
                    if tcb == 1:
                        # cos/sin ride after the second x tile: keeps the HBM
                        # burst at startup prioritized for the matmul pipeline
                        # (tcb0's RoPE DVE work tolerates the extra lag)
                        nc.vector.tensor_copy(cos[0:1, 0:1], xtc[0:1, 0:1])
                        nc.vector.tensor_copy(sin[0:1, 0:1], xtc[0:1, 0:1])
                        nc.gpsimd.dma_start(cos[:], COS[:])
                        nc.gpsimd.dma_start(sin[:], SIN[:])
                    qp = [qps.tile([128, 512], F32, tag=f"qp{t}",
                                   name=f"qp{t}") for t in range(2)]
                    kvp = kvps.tile([128, 512], F32, tag="kvp")
                    for cb in range(16):
                        st = dict(start=(cb == 0), stop=(cb == 15))
                        for t in range(2):
                            nc.tensor.matmul(
                                qp[t][:], wq_sb[cb][:, t * 128:(t + 1) * 128],
                                xt[cb], **st)
                        nc.tensor.matmul(kvp[:], wkv_sb[cb][:], xt[cb], **st)
                    # RoPE on q tiles
                    for t in range(2):
                        qraw = ptmp.tile([128, 512], BF, tag="qraw")
                        nc.vector.tensor_copy(qraw[:], qp[t][:])
                        rot = rtps.tile([128, 512], F32, tag="rot")
                        nc.tensor.matmul(rot[:], r2t[:], qraw[:],
                                         start=True, stop=True)
                        qsl_t = slice(t * TT + tcb * 512,
                                      t * TT + (tcb + 1) * 512)
                        nc.vector.tensor_mul(qTa[:, qsl_t], qraw[:],
                                             cos[:, tsl])
                        t2 = ptmp.tile([128, 512], BF, tag="t2")
                        nc.vector.tensor_mul(t2[:], rot[:], sin[:, tsl])
                        nc.vector.tensor_add(qTa[:, qsl_t], qTa[:, qsl_t],
                                             t2[:])
                    # v (rows 64:128 of kvp) -> natural layout via PE
                    vraw = ptmp.tile([128, 512], BF, tag="vraw")
                    nc.vector.tensor_copy(vraw[64:128, :], kvp[64:128, :])
                    for r in range(4):
                        i = tcb * 4 + r
                        tp = rtps.tile([128, 64], BF, tag="tp")
                        nc.tensor.transpose(
                            tp[:], vraw[64:128, r * 128:(r + 1) * 128],
                            ident[64:128, :])
                        nc.vector.tensor_copy(vp[:, i * VW:i * VW + 64], tp[:])
                        nc.vector.tensor_copy(
                            vp[:, i * VW + 64:i * VW + 65], onec[:])
                    # RoPE on k (rows 0:64 of kvp)
                    kraw = ptmp.tile([64, 512], BF, tag="kraw")
                    nc.vector.tensor_copy(kraw[:], kvp[0:64, :])
                    krot = rtps.tile([64, 512], F32, tag="rot")
                    nc.tensor.matmul(krot[:], r2t[0:64, 0:64], kraw[:],
                                     start=True, stop=True)
                    nc.vector.tensor_mul(kT[0:64, tsl], kraw[:],
                                          cos[0:64, tsl])
                    k2 = ptmp.tile([64, 512], BF, tag="k2")
                    nc.vector.tensor_mul(k2[:], krot[:], sin[0:64, tsl])
                    nc.vector.tensor_add(kT[0:64, tsl], kT[0:64, tsl], k2[:])
                    # mirror k to partitions 64:127 for row-packed scores
                    nc.sync.dma_start(kT[64:128, tsl], kT[0:64, tsl])

            # ================== Phase A: attention ==================
            groups = [list(range(N_CORES))]
            with tc.tile_pool(name="sc_ps", bufs=2, space="PSUM") as scps, \
                 tc.tile_pool(name="av_ps", bufs=2, space="PSUM") as avps, \
                 tc.tile_pool(name="ex_sb", bufs=4) as exp_pool, \
                 tc.tile_pool(name="sg_sb", bufs=4) as sg_pool:
                for qb in range(N_QB):
                    qsl = slice(qb * 256, (qb + 1) * 256)
                    g, rr = qb // 8, qb % 8
                    wb = qb % 8
                    base_kc = (qb // 8) * 16
                    nch = 2 * wb + 2
                    y_lo = avps.tile([65, 512], F32, tag="ylo")
                    y_hi = avps.tile([65, 512], F32, tag="yhi")
                    for ck in range(nch):
                        kc = base_kc + ck
                        ksl = slice(kc * 128, (kc + 1) * 128)
                        sc = scps.tile([128, 1024], F32, tag="sc")
                        qv = qTa[:].rearrange("p (t n) -> p t n", t=2)[:, :, qsl]
                        nc.tensor.matmul(sc[:, 0:512], kT[0:64, ksl],
                                         qv[0:64], start=True, stop=True)
                        nc.tensor.matmul(sc[:, 512:1024], kT[64:128, ksl],
                                         qv[64:128], start=True, stop=True)
                        ex = exp_pool.tile([128, 1024], BF, tag="ex")
                        nc.scalar.activation(ex[:], sc[:], AF.Exp, scale=0.125)
                        if ck == nch - 2:
                            for blk in range(4):
                                bsl = slice(blk * 256, (blk + 1) * 256)
                                nc.vector.tensor_mul(ex[:, bsl], ex[:, bsl],
                                                     ma[:])
                        if ck == nch - 1:
                            for blk in range(4):
                                bsl = slice(blk * 256, (blk + 1) * 256)
                                nc.vector.tensor_mul(ex[:, bsl], ex[:, bsl],
                                                     mb[:])
                        st = dict(start=(ck == 0), stop=(ck == nch - 1))
                        vsl = vp[:, kc * VW:kc * VW + 65]
                        nc.tensor.matmul(y_lo[:], vsl, ex[:, 0:512], **st)
                        nc.tensor.matmul(y_hi[:], vsl, ex[:, 512:1024], **st)
                    # stage this block's y + denominators straight to the
                    # a2a buffer (dest rr, chunk g); normalization happens on
                    # the receiver after the AllToAll
                    sg_lo = sg_pool.tile([65, 512], BF, tag="sglo")
                    sg_hi = sg_pool.tile([65, 512], BF, tag="sghi")
                    nc.vector.tensor_copy(sg_lo[:], y_lo[:])
                    nc.vector.tensor_copy(sg_hi[:], y_hi[:])
                    blocks = A2AI[g, rr, 0:256].rearrange(
                        "(c k p) q -> k p c q", c=2, k=2, p=64)
                    dens = A2AI[g, rr, 256:260].rearrange(
                        "(a b) q -> b a q", a=2, b=2)
                    nc.sync.dma_start(
                        blocks[0], sg_lo[0:64, :].rearrange(
                            "p (c q) -> p c q", c=2))
                    nc.gpsimd.dma_start(
                        blocks[1], sg_hi[0:64, :].rearrange(
                            "p (c q) -> p c q", c=2))
                    nc.sync.dma_start(
                        dens[0], sg_lo[64:65, :].rearrange(
                            "p (a q) -> p a q", a=2))
                    nc.gpsimd.dma_start(
                        dens[1], sg_hi[64:65, :].rearrange(
                            "p (a q) -> p a q", a=2))
                    # spread Wo preload over qb 0..11 so every tile lands
                    # before A2A#1; the tiny copy makes each load wait for
                    # this qb (8 SW DMA queues run concurrently, so without
                    # it the loads all issue at startup and steal HBM
                    # bandwidth from the x pipeline)
                    if qb < 12:
                        ccs = [qb] if qb < 8 else [8 + 2 * (qb - 8),
                                                   9 + 2 * (qb - 8)]
                        for cc in ccs:
                            nc.vector.tensor_copy(wob[cc][0:1, 0:1],
                                                  sg_lo[0:1, 0:1])
                            nc.gpsimd.dma_start(
                                wob[cc][:], WOB[cc * 128:(cc + 1) * 128, :])
                    if qb == 13:
                        # batch-0 y normalization, fully off the critical path
                        nc.gpsimd.dma_start(den[:, 0:256],
                                            A2AO[0, :, 256:260, :])
                        nc.vector.tensor_copy(den_f[:, 0:256], den[:, 0:256])
                        nc.vector.reciprocal(rec[:, 0:256], den_f[:, 0:256])
                        nc.vector.tensor_copy(rec_bf0[:], rec[:, 0:256])
                        rgv = rawg[0][:].rearrange("p (i t q) -> p i t q",
                                                   i=8, t=2)
                        for tb in range(2):
                            nc.gpsimd.dma_start(
                                rgv[:, :, tb, :],
                                A2AO[0, :, tb * 128:(tb + 1) * 128,
                                     :].rearrange("i p q -> p i q"))
                    if qb == 7:
                        nc.gpsimd.collective_compute(
                            "AllToAll", mybir.AluOpType.bypass,
                            replica_groups=groups,
                            ins=[A2AI[0]], outs=[A2AO[0]])
                nc.gpsimd.collective_compute(
                    "AllToAll", mybir.AluOpType.bypass,
                    replica_groups=groups, ins=[A2AI[1]], outs=[A2AO[1]])

            # ==================== Phase O: o_proj ====================
            with tc.tile_pool(name="o_ps", bufs=3, space="PSUM") as ops_pool, \
                 tc.tile_pool(name="bc_ps", bufs=2, space="PSUM") as bcps, \
                 tc.tile_pool(name="osb_sb", bufs=3) as osb_pool:
                nc.scalar.dma_start(den[:, 256:512], A2AO[1, :, 256:260, :])
                rgv = rawg[1][:].rearrange("p (i t q) -> p i t q", i=8, t=2)
                for tb in range(2):
                    nc.scalar.dma_start(
                        rgv[:, :, tb, :],
                        A2AO[1, :, tb * 128:(tb + 1) * 128, :].rearrange(
                            "i p q -> p i q"))
                rec_bf = osb_pool.tile([32, 512], BF, tag="recbf", bufs=1)
                nc.vector.tensor_copy(den_f[:, 256:512], den[:, 256:512])
                nc.vector.reciprocal(rec[:, 256:512], den_f[:, 256:512])
                nc.vector.tensor_copy(rec_bf[:, 256:512], rec[:, 256:512])
                for g in range(2):
                    gsl = slice(g * 256, (g + 1) * 256)
                    rsrc = rec_bf0 if g == 0 else rec_bf
                    for occ in range(16):
                        bc = bcps.tile([128, 256], F32, tag="bc")
                        nc.tensor.matmul(
                            bc[:], eall[:, occ * 128:(occ + 1) * 128],
                            rsrc[:, 0:256] if g == 0 else rec_bf[:, gsl],
                            start=True, stop=True)
                        nc.vector.scalar_tensor_tensor(
                            orhs[occ][:, gsl], bc[:], 1.0,
                            rawg[g][:, occ * 256:(occ + 1) * 256], MUL, MUL)
                # per-half m-loops: the batch-0 half starts right after the
                # last AV matmul, overlapping A2A#1 + batch-1 normalization
                for g in range(2):
                    gsl = slice(g * 256, (g + 1) * 256)
                    for m in range(16):
                        op = ops_pool.tile([128, 256], F32, tag="op")
                        for cc in range(16):
                            nc.tensor.matmul(
                                op[:], wob[cc][:, m * 128:(m + 1) * 128],
                                orhs[cc][:, gsl],
                                start=(cc == 0), stop=(cc == 15))
                        osb = osb_pool.tile([128, 256], F32, tag="osb")
                        nc.scalar.activation(osb[:], op[:], AF.Copy)
                        eng = nc.sync if m % 2 == 0 else nc.scalar
                        eng.dma_start(
                            OUTT[m * 128:(m + 1) * 128, gsl], osb[:])

    _split_waits(nc)
    return nc


def host_inputs(x, Wq, Wk, Wv, Wo):
    """Per-core input maps (host-side sharding + precomputed tables)."""
    x = np.asarray(x, np.float32)
    Wq = np.asarray(Wq, np.float32)
    Wk = np.asarray(Wk, np.float32)
    Wv = np.asarray(Wv, np.float32)
    Wo = np.asarray(Wo, np.float32)

    xt = np.ascontiguousarray(x.reshape(TT, C).T).astype(NPBF)   # [C, TT]

    inv_freq = (1.0 / (ROPE_THETA ** (np.arange(0, D, 2) / D))).astype(np.float64)
    pos = (np.arange(TT) % T).astype(np.float64)
    ang = pos[None, :] * inv_freq[np.arange(128) % 32][:, None]   # [128, TT]
    cos_t = np.cos(ang).astype(NPBF)
    sin_t = np.sin(ang).astype(NPBF)

    ki = np.arange(128)[:, None]
    qf = np.arange(256)[None, :]
    ma = (ki <= qf).astype(NPBF)
    mb = (ki + 128 <= qf).astype(NPBF)

    R = np.zeros((64, 64), np.float32)
    for mrow in range(32):
        R[mrow, mrow + 32] = -1.0
        R[mrow + 32, mrow] = 1.0
    R2 = np.zeros((128, 128), np.float32)
    R2[0:64, 0:64] = R
    R2[64:128, 64:128] = R
    r2t = np.ascontiguousarray(R2.T).astype(NPBF)

    wob = np.empty((C, C), np.float32)
    row = 0
    for i in range(N_CORES):
        for t in range(2):
            for h in (4 * i + t, 4 * i + t + 2):
                wob[row:row + 64, :] = Wo[h * 64:(h + 1) * 64, :]
                row += 64
    wob = wob.astype(NPBF)

    eall = np.zeros((32, 2048), np.float32)
    for cc in range(16):
        i, t = cc // 2, cc % 2
        eall[i * 4 + 2 * t, cc * 128:cc * 128 + 64] = 1.0
        eall[i * 4 + 2 * t + 1, cc * 128 + 64:cc * 128 + 128] = 1.0
    eall = eall.astype(NPBF)

    maps = []
    for c in range(N_CORES):
        wqs = np.empty((C, HL * D), np.float32)
        col = 0
        for t in range(2):
            for h in (4 * c + t, 4 * c + t + 2):
                wqs[:, col:col + 64] = Wq[:, h * 64:(h + 1) * 64]
                col += 64
        wkv = np.concatenate(
            [Wk[:, c * 64:(c + 1) * 64], Wv[:, c * 64:(c + 1) * 64]], axis=1)
        maps.append({
            "XT": xt,
            "WQS": wqs.astype(NPBF),
            "WKV": np.ascontiguousarray(wkv).astype(NPBF),
            "WOB": wob,
            "COS": cos_t,
            "SIN": sin_t,
            "MA": ma,
            "MB": mb,
            "ONEC": np.ones((128, 1), NPBF),
            "R2T": r2t,
            "IDN": np.concatenate([np.zeros((64, 64), np.float32),
                                   np.eye(64, dtype=np.float32)],
                                  axis=0).astype(NPBF),
            "EALL": eall,
        })
    return maps


def assemble_output(results, dtype=np.float32):
    out = np.empty((TT, C), dtype)
    for c in range(N_CORES):
        o = results[c]["OUTT"]  # [C, 512]; cols 0:256 = qb c, 256:512 = qb 8+c
        out[c * 256:(c + 1) * 256, :] = o[:, 0:256].T
        out[2048 + c * 256:2048 + (c + 1) * 256, :] = o[:, 256:512].T
    return out.reshape(B, T, C)


_NC_CACHE = None


def get_program():
    global _NC_CACHE
    if _NC_CACHE is None:
        _NC_CACHE = build_program()
    return _NC_CACHE


def kernel(x, Wq, Wk, Wv, Wo):
    nc = get_program()
    maps = host_inputs(x, Wq, Wk, Wv, Wo)
    res = run_bass_kernel_spmd(nc, maps, list(range(N_CORES)))
    return assemble_output(res.results, np.asarray(x).dtype)


if __name__ == "__main__":
    rng = np.random.default_rng(0)
    s = 1.0 / np.sqrt(C)
    x = rng.standard_normal((B, T, C), dtype=np.float32)
    Wq = rng.standard_normal((C, C), dtype=np.float32) * s
    Wk = rng.standard_normal((C, KV * D), dtype=np.float32) * s
    Wv = rng.standard_normal((C, KV * D), dtype=np.float32) * s
    Wo = rng.standard_normal((C, C), dtype=np.float32) * s
    y = kernel(x=x, Wq=Wq, Wk=Wk, Wv=Wv, Wo=Wo)
    print("out", y.shape, y.dtype, float(np.abs(y).max()))


# revision 18
# speedup vs baseline: 1.0052x; 1.0018x over previous
"""Causal self-attention (GQA + RoPE) on 8 Trainium2 NeuronCores.

Sharding: head-parallel. Core c owns heads [4c, 4c+4) — exactly one KV head —
over both batches flattened to 4096 rows. bf16 matmuls throughout (PSUM fp32
accumulate). Attention runs in transposed layout (keys on partitions, queries
on free dim; softmax denominators via a ones-column appended to V).

Output rows are interleaved across cores (dest r owns query blocks qb=r and
qb=8+r) so the y redistribution splits into two AllToAlls: the first (batch 0)
is issued mid-attention and overlaps with batch-1 attention compute; only the
second is (briefly) exposed. Wo is preloaded to SBUF during attention. Each
core then computes the full o_proj for its 512 interleaved rows.
"""

import numpy as np
import ml_dtypes

import concourse.bass as bass
import concourse.mybir as mybir
import concourse.tile as tile
from concourse.bass_utils import run_bass_kernel_spmd

F32 = mybir.dt.float32
BF = mybir.dt.bfloat16
AF = mybir.ActivationFunctionType
MUL = mybir.AluOpType.mult
NPBF = ml_dtypes.bfloat16

N_CORES = 8
B, T, C = 2, 2048, 2048
H, KV, D = 32, 8, 64
TT = B * T                     # 4096 flattened rows
HL = H // N_CORES              # 4 local heads
ROPE_THETA = 500000.0

N_TC = TT // 512               # 8 projection column chunks
N_QB = TT // 256               # 16 query blocks of 256
N_KC = TT // 128               # 32 key chunks of 128
VW = 68                        # padded per-chunk stride in the packed V tile


def _split_waits(nc):
    """This container's walrus accepts at most ONE sync-wait per instruction.

    Move extra waits onto NoOp carriers immediately before the instruction on
    the same engine (engine executes in order, so blocking semantics hold)."""
    for f in nc.m.functions:
        for blk in f.blocks:
            insts = list(blk.instructions)
            out = []
            changed = False
            for inst in insts:
                si = inst.sync_info
                if si is not None and len(si.on_wait) > 1:
                    changed = True
                    waits = list(si.on_wait)
                    for w in waits[:-1]:
                        nop = mybir.InstNoOp(
                            name=nc.get_next_instruction_name(), ins=[], outs=[]
                        )
                        nop.engine = inst.engine
                        nop.sync_info = mybir.SyncInfo(on_wait=[w], on_update=[])
                        out.append(nop)
                    inst.sync_info = mybir.SyncInfo(
                        on_wait=[waits[-1]], on_update=list(si.on_update)
                    )
                out.append(inst)
            if changed:
                blk.instructions = out


def build_program():
    nc = bass.Bass("TRN2", target_bir_lowering=False, debug=False,
                   num_devices=N_CORES)

    XT = nc.dram_tensor("XT", [C, TT], BF, kind="ExternalInput").ap()
    WQS = nc.dram_tensor("WQS", [C, HL * D], BF, kind="ExternalInput").ap()
    WKV = nc.dram_tensor("WKV", [C, 2 * D], BF, kind="ExternalInput").ap()
    WOB = nc.dram_tensor("WOB", [C, C], BF, kind="ExternalInput").ap()
    COS = nc.dram_tensor("COS", [128, TT], BF, kind="ExternalInput").ap()
    SIN = nc.dram_tensor("SIN", [128, TT], BF, kind="ExternalInput").ap()
    MA = nc.dram_tensor("MA", [128, 256], BF, kind="ExternalInput").ap()
    MB = nc.dram_tensor("MB", [128, 256], BF, kind="ExternalInput").ap()
    ONEC = nc.dram_tensor("ONEC", [128, 1], BF, kind="ExternalInput").ap()
    R2T = nc.dram_tensor("R2T", [128, 128], BF, kind="ExternalInput").ap()
    IDN = nc.dram_tensor("IDN", [128, 64], BF, kind="ExternalInput").ap()
    EALL = nc.dram_tensor("EALL", [32, 2048], BF, kind="ExternalInput").ap()
    OUTT = nc.dram_tensor("OUTT", [C, TT // N_CORES], F32,
                          kind="ExternalOutput").ap()

    DSI = nc.dram_tensor("cc_warm_in", [N_CORES, 16], BF).ap()
    DSO = nc.dram_tensor("cc_warm_out", [N_CORES, 16], BF).ap()
    A2AI = nc.dram_tensor("a2a_in", [2, N_CORES, HL * D + 4, 256], BF).ap()
    A2AO = nc.dram_tensor("a2a_out", [2, N_CORES, HL * D + 4, 256], BF).ap()

    with tile.TileContext(nc) as tc, nc.allow_low_precision(reason="bf16"):
        with tc.tile_pool(name="pers", bufs=1) as pers, \
             tc.tile_pool(name="wq_sb", bufs=1) as wq_pool, \
             tc.tile_pool(name="wo_sb", bufs=1) as wo_pool:
            # single big weight DMAs (per-queue outstanding limits serialize
            # many small loads); wq on sync, wkv on scalar, tables on gpsimd
            wqcat = wq_pool.tile([128, 16 * HL * D], BF)
            wqv = WQS.rearrange("(cb p) q -> p cb q", p=128)
            nc.sync.dma_start(
                wqcat[:].rearrange("p (cb q) -> p cb q", cb=16)[:, 0:2, :],
                wqv[:, 0:2, :])
            nc.sync.dma_start(
                wqcat[:].rearrange("p (cb q) -> p cb q", cb=16)[:, 2:16, :],
                wqv[:, 2:16, :])
            wq_sb = [wqcat[:, cb * HL * D:(cb + 1) * HL * D]
                     for cb in range(16)]
            wkvcat = wq_pool.tile([128, 16 * 2 * D], BF)
            nc.scalar.dma_start(
                wkvcat[:], WKV.rearrange("(cb p) q -> p cb q", p=128))
            wkv_sb = [wkvcat[:, cb * 2 * D:(cb + 1) * 2 * D]
                      for cb in range(16)]

            cos = pers.tile([128, TT], BF)
            sin = pers.tile([128, TT], BF)
            r2t = pers.tile([128, 128], BF)
            ident = pers.tile([128, 64], BF)
            onec = pers.tile([128, 1], BF)
            ma = pers.tile([128, 256], BF)
            mb = pers.tile([128, 256], BF)
            eall = pers.tile([32, 2048], BF)
            for dst, srcap in ((r2t, R2T), (ident, IDN), (onec, ONEC),
                               (ma, MA), (mb, MB), (eall, EALL)):
                nc.gpsimd.dma_start(dst[:], srcap[:])
            # tiny collective to absorb first-collective stream warmup
            nc.gpsimd.collective_compute(
                "AllToAll", mybir.AluOpType.bypass,
                replica_groups=[list(range(N_CORES))], ins=[DSI[:]],
                outs=[DSO[:]])

            qTa = pers.tile([128, 2 * TT], BF)
            kT = pers.tile([128, TT], BF)
            vp = pers.tile([128, N_KC * VW], BF)

            wob = [wo_pool.tile([128, C], BF, tag=f"wo{cc}", name=f"wo{cc}")
                   for cc in range(16)]
            den = wo_pool.tile([32, 512], BF)
            den_f = wo_pool.tile([32, 512], F32)
            rec = wo_pool.tile([32, 512], F32)
            orhs = [wo_pool.tile([128, 512], BF, tag=f"or{cc}",
                                 name=f"or{cc}") for cc in range(16)]
            rec_bf0 = wo_pool.tile([32, 256], BF)
            rawg = [wo_pool.tile([128, 16 * 256], BF, tag=f"rawg{g}",
                                 name=f"rawg{g}") for g in range(2)]

            # ============== Phase P: projections + RoPE ==============
            with tc.tile_pool(name="xt_sb", bufs=2) as xt_pool, \
                 tc.tile_pool(name="q_ps", bufs=2, space="PSUM") as qps, \
                 tc.tile_pool(name="kv_ps", bufs=2, space="PSUM") as kvps, \
                 tc.tile_pool(name="rt_ps", bufs=1, space="PSUM") as rtps, \
                 tc.tile_pool(name="proj_tmp", bufs=2) as ptmp:
                for tcb in range(N_TC):
                    tsl = slice(tcb * 512, (tcb + 1) * 512)
                    xtc = xt_pool.tile([128, 16 * 512], BF, tag="xtc",
                                       name="xtc")
                    for hf in range(2):
                        eng = nc.sync if hf == 0 else nc.scalar
                        eng.dma_start(
                            xtc[:, hf * 4096:(hf + 1) * 4096],
                            XT[hf * 1024:(hf + 1) * 1024, tsl].rearrange(
                                "(cb p) q -> p cb q", p=128))
                    xt = [xtc[:, cb * 512:(cb + 1) * 512] for cb in range(16)]
                    if tcb == 0:
                        # cos/sin ride after the first x tile: keeps the HBM
                        # burst at startup prioritized for the matmul pipeline
                        nc.vector.tensor_copy(cos[0:1, 0:1], xtc[0:1, 0:1])
                        nc.vector.tensor_copy(sin[0:1, 0:1], xtc[0:1, 0:1])
                        nc.gpsimd.dma_start(cos[:], COS[:])
                        nc.gpsimd.dma_start(sin[:], SIN[:])
                    qp = [qps.tile([128, 512], F32, tag=f"qp{t}",
                                   name=f"qp{t}") for t in range(2)]
                    kvp = kvps.tile([128, 512], F32, tag="kvp")
                    for cb in range(16):
                        st = dict(start=(cb == 0), stop=(cb == 15))
                        for t in range(2):
                            nc.tensor.matmul(
                                qp[t][:], wq_sb[cb][:, t * 128:(t + 1) * 128],
                                xt[cb], **st)
                        nc.tensor.matmul(kvp[:], wkv_sb[cb][:], xt[cb], **st)
                    # RoPE on q tiles
                    for t in range(2):
                        qraw = ptmp.tile([128, 512], BF, tag="qraw")
                        nc.vector.tensor_copy(qraw[:], qp[t][:])
                        rot = rtps.tile([128, 512], F32, tag="rot")
                        nc.tensor.matmul(rot[:], r2t[:], qraw[:],
                                         start=True, stop=True)
                        qsl_t = slice(t * TT + tcb * 512,
                                      t * TT + (tcb + 1) * 512)
                        nc.vector.tensor_mul(qTa[:, qsl_t], qraw[:],
                                             cos[:, tsl])
                        t2 = ptmp.tile([128, 512], BF, tag="t2")
                        nc.vector.tensor_mul(t2[:], rot[:], sin[:, tsl])
                        nc.vector.tensor_add(qTa[:, qsl_t], qTa[:, qsl_t],
                                             t2[:])
                    # v (rows 64:128 of kvp) -> natural layout via PE
                    vraw = ptmp.tile([128, 512], BF, tag="vraw")
                    nc.vector.tensor_copy(vraw[64:128, :], kvp[64:128, :])
                    for r in range(4):
                        i = tcb * 4 + r
                        tp = rtps.tile([128, 64], BF, tag="tp")
                        nc.tensor.transpose(
                            tp[:], vraw[64:128, r * 128:(r + 1) * 128],
                            ident[64:128, :])
                        nc.vector.tensor_copy(vp[:, i * VW:i * VW + 64], tp[:])
                        nc.vector.tensor_copy(
                            vp[:, i * VW + 64:i * VW + 65], onec[:])
                    # RoPE on k (rows 0:64 of kvp)
                    kraw = ptmp.tile([64, 512], BF, tag="kraw")
                    nc.vector.tensor_copy(kraw[:], kvp[0:64, :])
                    krot = rtps.tile([64, 512], F32, tag="rot")
                    nc.tensor.matmul(krot[:], r2t[0:64, 0:64], kraw[:],
                                     start=True, stop=True)
                    nc.vector.tensor_mul(kT[0:64, tsl], kraw[:],
                                          cos[0:64, tsl])
                    k2 = ptmp.tile([64, 512], BF, tag="k2")
                    nc.vector.tensor_mul(k2[:], krot[:], sin[0:64, tsl])
                    nc.vector.tensor_add(kT[0:64, tsl], kT[0:64, tsl], k2[:])
                    # mirror k to partitions 64:127 for row-packed scores
                    nc.sync.dma_start(kT[64:128, tsl], kT[0:64, tsl])

            # ================== Phase A: attention ==================
            groups = [list(range(N_CORES))]
            with tc.tile_pool(name="sc_ps", bufs=2, space="PSUM") as scps, \
                 tc.tile_pool(name="av_ps", bufs=2, space="PSUM") as avps, \
                 tc.tile_pool(name="ex_sb", bufs=4) as exp_pool, \
                 tc.tile_pool(name="sg_sb", bufs=4) as sg_pool:
                for qb in range(N_QB):
                    qsl = slice(qb * 256, (qb + 1) * 256)
                    g, rr = qb // 8, qb % 8
                    wb = qb % 8
                    base_kc = (qb // 8) * 16
                    nch = 2 * wb + 2
                    y_lo = avps.tile([65, 512], F32, tag="ylo")
                    y_hi = avps.tile([65, 512], F32, tag="yhi")
                    for ck in range(nch):
                        kc = base_kc + ck
                        ksl = slice(kc * 128, (kc + 1) * 128)
                        sc = scps.tile([128, 1024], F32, tag="sc")
                        qv = qTa[:].rearrange("p (t n) -> p t n", t=2)[:, :, qsl]
                        nc.tensor.matmul(sc[:, 0:512], kT[0:64, ksl],
                                         qv[0:64], start=True, stop=True)
                        nc.tensor.matmul(sc[:, 512:1024], kT[64:128, ksl],
                                         qv[64:128], start=True, stop=True)
                        ex = exp_pool.tile([128, 1024], BF, tag="ex")
                        nc.scalar.activation(ex[:], sc[:], AF.Exp, scale=0.125)
                        if ck == nch - 2:
                            for blk in range(4):
                                bsl = slice(blk * 256, (blk + 1) * 256)
                                nc.vector.tensor_mul(ex[:, bsl], ex[:, bsl],
                                                     ma[:])
                        if ck == nch - 1:
                            for blk in range(4):
                                bsl = slice(blk * 256, (blk + 1) * 256)
                                nc.vector.tensor_mul(ex[:, bsl], ex[:, bsl],
                                                     mb[:])
                        st = dict(start=(ck == 0), stop=(ck == nch - 1))
                        vsl = vp[:, kc * VW:kc * VW + 65]
                        nc.tensor.matmul(y_lo[:], vsl, ex[:, 0:512], **st)
                        nc.tensor.matmul(y_hi[:], vsl, ex[:, 512:1024], **st)
                    # stage this block's y + denominators straight to the
                    # a2a buffer (dest rr, chunk g); normalization happens on
                    # the receiver after the AllToAll
                    sg_lo = sg_pool.tile([65, 512], BF, tag="sglo")
                    sg_hi = sg_pool.tile([65, 512], BF, tag="sghi")
                    nc.vector.tensor_copy(sg_lo[:], y_lo[:])
                    nc.vector.tensor_copy(sg_hi[:], y_hi[:])
                    blocks = A2AI[g, rr, 0:256].rearrange(
                        "(c k p) q -> k p c q", c=2, k=2, p=64)
                    dens = A2AI[g, rr, 256:260].rearrange(
                        "(a b) q -> b a q", a=2, b=2)
                    nc.sync.dma_start(
                        blocks[0], sg_lo[0:64, :].rearrange(
                            "p (c q) -> p c q", c=2))
                    nc.gpsimd.dma_start(
                        blocks[1], sg_hi[0:64, :].rearrange(
                            "p (c q) -> p c q", c=2))
                    nc.sync.dma_start(
                        dens[0], sg_lo[64:65, :].rearrange(
                            "p (a q) -> p a q", a=2))
                    nc.gpsimd.dma_start(
                        dens[1], sg_hi[64:65, :].rearrange(
                            "p (a q) -> p a q", a=2))
                    # spread Wo preload over qb 0..11 so every tile lands
                    # before A2A#1; the tiny copy makes each load wait for
                    # this qb (8 SW DMA queues run concurrently, so without
                    # it the loads all issue at startup and steal HBM
                    # bandwidth from the x pipeline)
                    if qb < 12:
                        ccs = [qb] if qb < 8 else [8 + 2 * (qb - 8),
                                                   9 + 2 * (qb - 8)]
                        for cc in ccs:
                            nc.vector.tensor_copy(wob[cc][0:1, 0:1],
                                                  sg_lo[0:1, 0:1])
                            nc.gpsimd.dma_start(
                                wob[cc][:], WOB[cc * 128:(cc + 1) * 128, :])
                    if qb == 13:
                        # batch-0 y normalization, fully off the critical path
                        nc.gpsimd.dma_start(den[:, 0:256],
                                            A2AO[0, :, 256:260, :])
                        nc.vector.tensor_copy(den_f[:, 0:256], den[:, 0:256])
                        nc.vector.reciprocal(rec[:, 0:256], den_f[:, 0:256])
                        nc.vector.tensor_copy(rec_bf0[:], rec[:, 0:256])
                        rgv = rawg[0][:].rearrange("p (i t q) -> p i t q",
                                                   i=8, t=2)
                        for tb in range(2):
                            nc.gpsimd.dma_start(
                                rgv[:, :, tb, :],
                                A2AO[0, :, tb * 128:(tb + 1) * 128,
                                     :].rearrange("i p q -> p i q"))
                    if qb == 7:
                        nc.gpsimd.collective_compute(
                            "AllToAll", mybir.AluOpType.bypass,
                            replica_groups=groups,
                            ins=[A2AI[0]], outs=[A2AO[0]])
                nc.gpsimd.collective_compute(
                    "AllToAll", mybir.AluOpType.bypass,
                    replica_groups=groups, ins=[A2AI[1]], outs=[A2AO[1]])

            # ==================== Phase O: o_proj ====================
            with tc.tile_pool(name="o_ps", bufs=3, space="PSUM") as ops_pool, \
                 tc.tile_pool(name="bc_ps", bufs=2, space="PSUM") as bcps, \
                 tc.tile_pool(name="osb_sb", bufs=3) as osb_pool:
                nc.scalar.dma_start(den[:, 256:512], A2AO[1, :, 256:260, :])
                rgv = rawg[1][:].rearrange("p (i t q) -> p i t q", i=8, t=2)
                for tb in range(2):
                    nc.scalar.dma_start(
                        rgv[:, :, tb, :],
                        A2AO[1, :, tb * 128:(tb + 1) * 128, :].rearrange(
                            "i p q -> p i q"))
                rec_bf = osb_pool.tile([32, 512], BF, tag="recbf", bufs=1)
                nc.vector.tensor_copy(den_f[:, 256:512], den[:, 256:512])
                nc.vector.reciprocal(rec[:, 256:512], den_f[:, 256:512])
                nc.vector.tensor_copy(rec_bf[:, 256:512], rec[:, 256:512])
                for g in range(2):
                    gsl = slice(g * 256, (g + 1) * 256)
                    rsrc = rec_bf0 if g == 0 else rec_bf
                    for occ in range(16):
                        bc = bcps.tile([128, 256], F32, tag="bc")
                        nc.tensor.matmul(
                            bc[:], eall[:, occ * 128:(occ + 1) * 128],
                            rsrc[:, 0:256] if g == 0 else rec_bf[:, gsl],
                            start=True, stop=True)
                        nc.vector.scalar_tensor_tensor(
                            orhs[occ][:, gsl], bc[:], 1.0,
                            rawg[g][:, occ * 256:(occ + 1) * 256], MUL, MUL)
                # per-half m-loops: the batch-0 half starts right after the
                # last AV matmul, overlapping A2A#1 + batch-1 normalization
                for g in range(2):
                    gsl = slice(g * 256, (g + 1) * 256)
                    for m in range(16):
                        op = ops_pool.tile([128, 256], F32, tag="op")
                        for cc in range(16):
                            nc.tensor.matmul(
                                op[:], wob[cc][:, m * 128:(m + 1) * 128],
                                orhs[cc][:, gsl],
                                start=(cc == 0), stop=(cc == 15))
                        osb = osb_pool.tile([128, 256], F32, tag="osb")
                        nc.scalar.activation(osb[:], op[:], AF.Copy)
                        eng = nc.sync if m % 2 == 0 else nc.scalar
                        eng.dma_start(
                            OUTT[m * 128:(m + 1) * 128, gsl], osb[:])

    _split_waits(nc)
    return nc


def host_inputs(x, Wq, Wk, Wv, Wo):
    """Per-core input maps (host-side sharding + precomputed tables)."""
    x = np.asarray(x, np.float32)
    Wq = np.asarray(Wq, np.float32)
    Wk = np.asarray(Wk, np.float32)
    Wv = np.asarray(Wv, np.float32)
    Wo = np.asarray(Wo, np.float32)

    xt = np.ascontiguousarray(x.reshape(TT, C).T).astype(NPBF)   # [C, TT]

    inv_freq = (1.0 / (ROPE_THETA ** (np.arange(0, D, 2) / D))).astype(np.float64)
    pos = (np.arange(TT) % T).astype(np.float64)
    ang = pos[None, :] * inv_freq[np.arange(128) % 32][:, None]   # [128, TT]
    cos_t = np.cos(ang).astype(NPBF)
    sin_t = np.sin(ang).astype(NPBF)

    ki = np.arange(128)[:, None]
    qf = np.arange(256)[None, :]
    ma = (ki <= qf).astype(NPBF)
    mb = (ki + 128 <= qf).astype(NPBF)

    R = np.zeros((64, 64), np.float32)
    for mrow in range(32):
        R[mrow, mrow + 32] = -1.0
        R[mrow + 32, mrow] = 1.0
    R2 = np.zeros((128, 128), np.float32)
    R2[0:64, 0:64] = R
    R2[64:128, 64:128] = R
    r2t = np.ascontiguousarray(R2.T).astype(NPBF)

    wob = np.empty((C, C), np.float32)
    row = 0
    for i in range(N_CORES):
        for t in range(2):
            for h in (4 * i + t, 4 * i + t + 2):
                wob[row:row + 64, :] = Wo[h * 64:(h + 1) * 64, :]
                row += 64
    wob = wob.astype(NPBF)

    eall = np.zeros((32, 2048), np.float32)
    for cc in range(16):
        i, t = cc // 2, cc % 2
        eall[i * 4 + 2 * t, cc * 128:cc * 128 + 64] = 1.0
        eall[i * 4 + 2 * t + 1, cc * 128 + 64:cc * 128 + 128] = 1.0
    eall = eall.astype(NPBF)

    maps = []
    for c in range(N_CORES):
        wqs = np.empty((C, HL * D), np.float32)
        col = 0
        for t in range(2):
            for h in (4 * c + t, 4 * c + t + 2):
                wqs[:, col:col + 64] = Wq[:, h * 64:(h + 1) * 64]
                col += 64
        wkv = np.concatenate(
            [Wk[:, c * 64:(c + 1) * 64], Wv[:, c * 64:(c + 1) * 64]], axis=1)
        maps.append({
            "XT": xt,
            "WQS": wqs.astype(NPBF),
            "WKV": np.ascontiguousarray(wkv).astype(NPBF),
            "WOB": wob,
            "COS": cos_t,
            "SIN": sin_t,
            "MA": ma,
            "MB": mb,
            "ONEC": np.ones((128, 1), NPBF),
            "R2T": r2t,
            "IDN": np.concatenate([np.zeros((64, 64), np.float32),
                                   np.eye(64, dtype=np.float32)],
                                  axis=0).astype(NPBF),
            "EALL": eall,
        })
    return maps


def assemble_output(results, dtype=np.float32):
    out = np.empty((TT, C), dtype)
    for c in range(N_CORES):
        o = results[c]["OUTT"]  # [C, 512]; cols 0:256 = qb c, 256:512 = qb 8+c
        out[c * 256:(c + 1) * 256, :] = o[:, 0:256].T
        out[2048 + c * 256:2048 + (c + 1) * 256, :] = o[:, 256:512].T
    return out.reshape(B, T, C)


_NC_CACHE = None


def get_program():
    global _NC_CACHE
    if _NC_CACHE is None:
        _NC_CACHE = build_program()
    return _NC_CACHE


def kernel(x, Wq, Wk, Wv, Wo):
    nc = get_program()
    maps = host_inputs(x, Wq, Wk, Wv, Wo)
    res = run_bass_kernel_spmd(nc, maps, list(range(N_CORES)))
    return assemble_output(res.results, np.asarray(x).dtype)


if __name__ == "__main__":
    rng = np.random.default_rng(0)
    s = 1.0 / np.sqrt(C)
    x = rng.standard_normal((B, T, C), dtype=np.float32)
    Wq = rng.standard_normal((C, C), dtype=np.float32) * s
    Wk = rng.standard_normal((C, KV * D), dtype=np.float32) * s
    Wv = rng.standard_normal((C, KV * D), dtype=np.float32) * s
    Wo = rng.standard_normal((C, C), dtype=np.float32) * s
    y = kernel(x=x, Wq=Wq, Wk=Wk, Wv=Wv, Wo=Wo)
    print("out", y.shape, y.dtype, float(np.abs(y).max()))
